# revision 28
# baseline (speedup 1.0000x reference)
"""AttentionTeacher Trainium2 kernel (fp8 DoubleRow projections +
engine-rebalanced softmax pipeline).

Math (reference):
    q = query @ Wq.T + bq;  k = key @ Wk.T + bk          [B,S,HID]
    per head h (HD=64): scores_h = q_h k_h^T / 8 + mask  [B,NH,S,S]
    probs_h = softmax(scores_h)
    out = (sum_h probs_h) @ V / NH                       [B,S,HID]

Sharding: 8 cores, SPMD, no collectives. Core i handles batch b=i//2 and
query rows [512*(i%2), 512*(i%2+1)). The K-projection is duplicated
across the pair of cores sharing a batch (cheap in fp8: ~3.4us of PE).

The critical engine is ACT: 64 exp([128,1024]) ops with accum_out row
sums are ~78us and exp runs nowhere else, so ACT does *only* exp (plus
evacuations for t=0 during the DMA ramp and the final qblock's tail
copies, both in ACT idle windows). Everything else is balanced around
that wall (TimelineSim busy, per core):
  PE   ~60us: fp8e4 DoubleRow projections (2 k-planes/instr, 0.5
       cyc/col), bf16 scores (K=64 via tile_position row pairs), fp32
       transposes of P, bf16 P^T @ V.
  DVE  ~77us: all PSUM evacuations (tensor_scalar with scale+bias
       folded), even-head chain STTs, odd-head prescales, reciprocals,
       qb<3 copies, the qb3 tail merge.
  Pool ~62us: odd-head chain adds (plain TensorTensor: GPSIMD can read
       neither PSUM nor run TensorScalarPtr on real HW), qb<3 merges.

fp8 notes: W is staged *32 (avoids e4m3 denormals at sigma=0.02); the
1/32 (and the 1/8 score scale on the q side) is folded into the
evacuation. Scores/PV stay bf16: fp8 P would sit in the denormal range
and the extra score error would eat the 2e-2 budget (measured rel err
6.4e-3 vs 2.5e-3 all-bf16).

Schedule notes (emission order ~ per-engine execution order):
  - One serial HBM stream in the model: DMAs are criticality-ordered
    (biases, tile-0 W, kin, qin, W tiles 1-7, ident, V) and sized so the
    ~650ns/DMA fixed cost stays amortized; K-projection is emitted
    before Q so PE follows the DMA arrival order.
  - Chains lag exps by one head pair (per-pair reciprocals); the final
    pair gets per-head reciprocals so chain 14 overlaps exp 15, and
    qb3's Pool sub-chain ends at h13 so only h15's STT and the halved
    DVE merge separate the last exp from the transposes.
  - Each phase prefetches the next qblock's first head pair ahead of the
    end-of-phase pace work; qb0/qb1/qb2 outputs are stage-split
    (transposes / PV half / PV half) between head pairs, with qb2's last
    PV half after the final exp to keep PE p-state warm into qb3's PV.
  - pp/ov live in separate single-buffer PSUM pools so the scheduler
    doesn't serialize qb2's ready PV behind qb3's transposes.
"""

import numpy as np
import ml_dtypes

import concourse.bass as bass
import concourse.tile as tile
from concourse import bacc, mybir
from concourse.bass_utils import run_bass_kernel_spmd

N_CORES = 8
B, S, HID, NH, HD = 4, 1024, 1024, 16, 64
SQ = S // 2          # query rows per core
QB = SQ // 128       # query blocks per core
DT = HID // 128      # dout tiles (2 heads each)
KTI = HID // 128     # contraction (din) tiles
CD = mybir.dt.bfloat16
F8 = mybir.dt.float8e4
F32 = mybir.dt.float32
BF16_NP = ml_dtypes.bfloat16
F8_NP = mybir.dt.np(F8)

W_SCALE = 32.0       # host-side W upscale (fp8 denormal avoidance)
Q_EVAC = 1.0 / (W_SCALE * 8.0)   # un-scale + 1/sqrt(HD)
K_EVAC = 1.0 / W_SCALE

_ts = bass.ts
_mult = mybir.AluOpType.mult
_add = mybir.AluOpType.add
_EXP = mybir.ActivationFunctionType.Exp
_IDENT = mybir.ActivationFunctionType.Identity
_DR = mybir.MatmulPerfMode.DoubleRow

_CACHE: dict = {}

# Real-HW engine limits (BIR verifier): Pool/GPSIMD cannot read PSUM and
# cannot run TensorScalarPtr. So: PSUM evacuations live on DVE (plus the
# first tiles on ACT, which is otherwise idle during the DMA ramp), and
# Pool chain heads are fed by a DVE prescale (E *= 1/Z in place) followed
# by a plain Pool TensorTensor add.
_DVE_HEADS = frozenset(range(0, NH, 2))  # one Pool head per pair
_ACT_EVAC_T = frozenset({0})


def _build_program(reps: int = 1):
    nc = bacc.Bacc(
        "TRN2", target_bir_lowering=False, debug=False, num_devices=N_CORES
    )
    d_q8 = nc.dram_tensor("q8_in", [128, KTI * SQ], F8, kind="ExternalInput")
    d_k8 = nc.dram_tensor("k8_in", [128, KTI * S], F8, kind="ExternalInput")
    # W interleaved per dout tile: block t = [wq_t | wk_t], each [128, KTI*128]
    d_w8 = nc.dram_tensor(
        "w8_in", [128, 2 * KTI * HID], F8, kind="ExternalInput"
    )
    d_v = nc.dram_tensor("v_in", [128, KTI * HID], CD, kind="ExternalInput")
    # consts merged: bq [0:DT], bk [DT:2DT], ident [2DT:2DT+128]
    d_c = nc.dram_tensor("const_in", [128, 2 * DT + 128], F32, kind="ExternalInput")
    d_o = nc.dram_tensor("o_out", [SQ, HID], F32, kind="ExternalOutput")

    with tile.TileContext(nc) as tc:
        with (
            tc.tile_pool(name="const", bufs=1) as const_pool,
            tc.tile_pool(name="xin", bufs=1) as xin_pool,
            tc.tile_pool(name="proj", bufs=1) as proj_pool,
            tc.tile_pool(name="e", bufs=16) as e_pool,
            tc.tile_pool(name="ch", bufs=8) as ch_pool,
            tc.tile_pool(name="z", bufs=20) as z_pool,
            tc.tile_pool(name="pt", bufs=2) as pt_pool,
            tc.tile_pool(name="osb", bufs=2) as o_pool,
            tc.tile_pool(name="proj_ps", bufs=2, space="PSUM") as proj_ps,
            tc.tile_pool(name="sc_ps", bufs=2, space="PSUM") as sc_ps,
            tc.tile_pool(name="pp_ps", bufs=1, space="PSUM") as pp_ps,
            tc.tile_pool(name="ov_ps", bufs=1, space="PSUM") as ov_ps,
        ):
          for _rep in range(reps):
            # ---- input DMAs (criticality-ordered; big transfers so the
            # per-DMA HWDGE fixed cost (~650ns) doesn't dominate) ----
            w_sb = xin_pool.tile([128, 2 * KTI * HID], F8, tag="w8", name="w8")
            qin_sb = xin_pool.tile([128, KTI * SQ], F8, tag="q8", name="q8")
            kin_sb = xin_pool.tile([128, KTI * S], F8, tag="k8", name="k8")
            c_sb = const_pool.tile([128, 2 * DT + 128], F32, tag="c", name="c_sb")
            nc.sync.dma_start(c_sb[:], d_c.ap()[:])
            nc.sync.dma_start(
                w_sb[:, _ts(0, 2 * HID)], d_w8.ap()[:, _ts(0, 2 * HID)]
            )
            nc.sync.dma_start(kin_sb[:], d_k8.ap()[:])
            nc.sync.dma_start(qin_sb[:], d_q8.ap()[:])
            for t in range(1, DT):
                nc.sync.dma_start(
                    w_sb[:, _ts(t, 2 * HID)], d_w8.ap()[:, _ts(t, 2 * HID)]
                )
            v_sb = xin_pool.tile([128, KTI * HID], CD, tag="v", name="v_sb")
            nc.sync.dma_start(v_sb[:], d_v.ap()[:])

            bq_sb = c_sb[:, 0:DT]
            bk_sb = c_sb[:, DT : 2 * DT]
            ident = c_sb[:, 2 * DT : 2 * DT + 128]
            # per-tile [128, KTI, 128] views of W; [128, KTI, cols] of q/k/v
            wq3 = [
                w_sb[:, _ts(2 * t, HID)].rearrange("p (c f) -> p c f", c=KTI)
                for t in range(DT)
            ]
            wk3 = [
                w_sb[:, _ts(2 * t + 1, HID)].rearrange("p (c f) -> p c f", c=KTI)
                for t in range(DT)
            ]
            qin3 = qin_sb[:].rearrange("p (c f) -> p c f", c=KTI)
            kin3 = kin_sb[:].rearrange("p (c f) -> p c f", c=KTI)
            v3 = v_sb[:].rearrange("p (c f) -> p c f", c=KTI)

            # preload the ACT exp table while DMAs run
            warm = const_pool.tile([128, 1], F32, tag="warm", name="warm")
            nc.gpsimd.memset(warm[:], 0.0)
            warm2 = const_pool.tile([128, 1], F32, tag="warm2", name="warm2")
            nc.scalar.activation(warm2[:], warm[:], _EXP)

            qt = [
                proj_pool.tile([128, SQ], CD, tag=f"qt{t}", name=f"qt{t}")
                for t in range(DT)
            ]
            ktp = [
                proj_pool.tile([128, S], CD, tag=f"kt{t}", name=f"ktp{t}")
                for t in range(DT)
            ]

            # ---- per-qblock attention state ----
            zts = {}     # qb -> [128, NH] f32 row sums
            es = {}      # (qb, h) -> E tile (f32)
            invz = {}    # (qb, h) -> [128, 1] f32 reciprocal column
            chain_a = {}  # qb -> DVE sub-chain tile
            chain_b = {}  # qb -> Pool sub-chain tile
            merged = {}  # qb -> P tile

            def emit_proj(t):
                # K first: kin is DMA'd before qin, and PE runs in FIFO order
                for nh in range(2):
                    ps2 = proj_ps.tile([128, 512], F32, tag="proj", name="proj_k_ps")
                    for j in range(KTI // 2):
                        nc.tensor.matmul(
                            ps2[:],
                            wk3[t][:, 2 * j : 2 * j + 2, :],
                            kin3[:, 2 * j : 2 * j + 2, _ts(nh, 512)],
                            start=(j == 0), stop=(j == KTI // 2 - 1),
                            perf_mode=_DR,
                        )
                    if t in _ACT_EVAC_T:
                        nc.scalar.activation(
                            ktp[t][:, _ts(nh, 512)], ps2[:], _IDENT,
                            bias=bk_sb[:, t : t + 1], scale=K_EVAC,
                        )
                    else:
                        nc.vector.tensor_scalar(
                            out=ktp[t][:, _ts(nh, 512)], in0=ps2[:], scalar1=K_EVAC,
                            scalar2=bk_sb[:, t : t + 1], op0=_mult, op1=_add,
                        )
                ps = proj_ps.tile([128, SQ], F32, tag="proj", name="proj_q_ps")
                for j in range(KTI // 2):
                    nc.tensor.matmul(
                        ps[:],
                        wq3[t][:, 2 * j : 2 * j + 2, :],
                        qin3[:, 2 * j : 2 * j + 2, :],
                        start=(j == 0), stop=(j == KTI // 2 - 1),
                        perf_mode=_DR,
                    )
                if t in _ACT_EVAC_T:
                    nc.scalar.activation(
                        qt[t][:], ps[:], _IDENT,
                        bias=bq_sb[:, t : t + 1], scale=Q_EVAC,
                    )
                else:
                    nc.vector.tensor_scalar(
                        out=qt[t][:], in0=ps[:], scalar1=Q_EVAC,
                        scalar2=bq_sb[:, t : t + 1], op0=_mult, op1=_add,
                    )

            def emit_head(qb, h):
                if qb not in zts:
                    zts[qb] = z_pool.tile([128, NH], F32, tag="z", name="zt")
                t, half = h // 2, h % 2
                d0 = 64 * half
                sc = sc_ps.tile([128, S], F32, tag="sc", name="sc")
                for n2 in range(2):
                    nc.tensor.matmul(
                        sc[:, _ts(n2, 512)],
                        qt[t][d0 : d0 + 64, _ts(qb, 128)],
                        ktp[t][d0 : d0 + 64, _ts(n2, 512)],
                        start=True, stop=True, tile_position=(d0, 0),
                    )
                e = e_pool.tile([128, S], F32, tag="e", name="e")
                nc.scalar.activation(e[:], sc[:], _EXP, accum_out=zts[qb][:, h : h + 1])
                es[(qb, h)] = e

            def emit_recip(qb, h0, nh):
                # inverse of Z for heads [h0, h0+nh) right after their exps
                inv = z_pool.tile([128, nh], F32, tag="z", name="inv_z")
                nc.vector.reciprocal(inv[:], zts[qb][:, h0 : h0 + nh])
                for k in range(nh):
                    invz[(qb, h0 + k)] = inv[:, k : k + 1]

            def emit_chain(qb, h):
                # DVE-native heads: one STT. Pool heads: DVE prescale of E
                # in place, then a plain TT add on Pool.
                sv = invz[(qb, h)]
                e = es[(qb, h)]
                on_dve = h in _DVE_HEADS or (qb == 3 and h == 15)
                book = chain_a if on_dve else chain_b
                prev = book.get(qb)
                if on_dve:
                    nxt = ch_pool.tile([128, S], F32, tag="ch", name="chain")
                    if prev is None:
                        nc.vector.tensor_scalar(
                            out=nxt[:], in0=e[:], scalar1=sv,
                            scalar2=None, op0=_mult,
                        )
                    else:
                        nc.vector.scalar_tensor_tensor(
                            out=nxt[:], in0=e[:], scalar=sv,
                            in1=prev[:], op0=_mult, op1=_add,
                        )
                    book[qb] = nxt
                else:
                    nc.vector.tensor_scalar(
                        out=e[:], in0=e[:], scalar1=sv, scalar2=None, op0=_mult
                    )
                    if prev is None:
                        book[qb] = e  # scaled E doubles as the B seed
                    else:
                        nxt = ch_pool.tile([128, S], F32, tag="ch", name="chain")
                        nc.gpsimd.tensor_tensor(
                            out=nxt[:], in0=e[:], in1=prev[:], op=_add
                        )
                        book[qb] = nxt

            def emit_merge(qb):
                out = ch_pool.tile([128, S], F32, tag="ch", name="pmerged")
                if qb < 3:
                    nc.gpsimd.tensor_tensor(
                        out=out[:], in0=chain_a[qb][:], in1=chain_b[qb][:], op=_add
                    )
                else:
                    # tail merge on DVE in halves (qb3's Pool B-chain ends
                    # at h13, so only h15's STT separates exp15 from this)
                    for c in range(2):
                        nc.vector.tensor_tensor(
                            out=out[:, _ts(c, 512)],
                            in0=chain_a[3][:, _ts(c, 512)],
                            in1=chain_b[3][:, _ts(c, 512)], op=_add,
                        )
                merged[qb] = out

            def emit_pace(qb, i):
                # chains lag exps by one pair; final pair gets per-head
                # recips so chain 14 overlaps exp 15
                if i == 7:
                    emit_recip(qb, 14, 1)
                    emit_chain(qb, 12)
                    emit_chain(qb, 13)
                    emit_chain(qb, 14)
                    emit_recip(qb, 15, 1)
                    emit_chain(qb, 15)
                    emit_merge(qb)
                    return
                emit_recip(qb, 2 * i, 2)
                if i >= 1:
                    emit_chain(qb, 2 * i - 2)
                    emit_chain(qb, 2 * i - 1)

            out_state = {}

            def emit_out_stage(qb, stage):
                # staged so each piece of PE work interleaves between head
                # pairs instead of starving the exp stream
                if stage == 0:
                    # PE transposes of P (f32) -> bf16 P^T via DVE/Pool
                    p = merged[qb]
                    ptsb = pt_pool.tile([128, S], CD, tag="pt", name="ptsb")
                    for c in range(2):
                        pp = pp_ps.tile([128, 512], F32, tag="pp", name="pp")
                        for j in range(4):
                            nc.tensor.transpose(
                                pp[:, _ts(j, 128)], p[:, _ts(4 * c + j, 128)],
                                ident,
                            )
                        if qb == 3:
                            nc.scalar.copy(ptsb[:, _ts(c, 512)], pp[:])
                        else:
                            nc.vector.tensor_copy(ptsb[:, _ts(c, 512)], pp[:])
                    osb = o_pool.tile([128, HID], F32, tag="osb", name="osb")
                    out_state[qb] = (ptsb, osb)
                else:
                    n2 = stage - 1
                    ptsb, osb = out_state[qb]
                    ov = ov_ps.tile([128, 512], F32, tag="ov", name="ov")
                    for kt_i in range(KTI):
                        nc.tensor.matmul(
                            ov[:], ptsb[:, _ts(kt_i, 128)],
                            v3[:, kt_i, _ts(n2, 512)],
                            start=(kt_i == 0), stop=(kt_i == KTI - 1),
                        )
                    if qb == 3:
                        nc.scalar.copy(osb[:, _ts(n2, 512)], ov[:])
                    else:
                        nc.vector.tensor_copy(osb[:, _ts(n2, 512)], ov[:])
                    nc.sync.dma_start(
                        d_o.ap()[_ts(qb, 128), _ts(n2, 512)], osb[:, _ts(n2, 512)]
                    )

            def emit_out(qb):
                for stage in range(3):
                    emit_out_stage(qb, stage)

            # ---- emission schedule ----
            # t-loop: heads for tile t, next tile's projections (so their
            # evacuations precede chain ops in the DVE/Pool queues), pacing.
            # Each phase prefetches the next qblock's first head pair ahead
            # of the heavy end-of-phase pace (chains+merge) so the exp
            # stream doesn't stall at the transition.
            emit_proj(0)
            for t in range(DT):
                for qb in (0, 1):
                    emit_head(qb, 2 * t)
                    emit_head(qb, 2 * t + 1)
                if t + 1 < DT:
                    emit_proj(t + 1)
                if t == DT - 1:
                    emit_head(2, 0)
                    emit_head(2, 1)
                emit_pace(0, t)
                emit_pace(1, t)
            # qb2 phase (pair i emitted during pace i-1 slot)
            out_sched_2 = {2: (0, 0), 3: (0, 1), 4: (0, 2), 5: (1, 0), 6: (1, 1)}
            for i in range(1, 8):
                emit_head(2, 2 * i)
                emit_head(2, 2 * i + 1)
                emit_pace(2, i - 1)
                if i - 1 in out_sched_2:
                    emit_out_stage(*out_sched_2[i - 1])
            emit_head(3, 0)
            emit_head(3, 1)
            emit_pace(2, 7)
            emit_out_stage(1, 2)
            # qb3 phase: qb2's PV held late so PE stays warm into the tail
            out_sched_3 = {5: (2, 0), 6: (2, 1)}
            for i in range(1, 8):
                emit_head(3, 2 * i)
                emit_head(3, 2 * i + 1)
                emit_pace(3, i - 1)
                if i - 1 in out_sched_3:
                    emit_out_stage(*out_sched_3[i - 1])
            emit_pace(3, 7)
            emit_out_stage(2, 2)
            emit_out(3)

    nc.compile()
    return nc


def _get_program(reps: int = 1):
    key = f"nc{reps}"
    if key not in _CACHE:
        _CACHE[key] = _build_program(reps)
    return _CACHE[key]


class _Runner:
    """Compile-once SPMD executor (mirrors run_bass_via_pjrt's multi-core
    path, but keeps the jitted function so repeat calls skip re-compile)."""

    def __init__(self, nc):
        import jax
        from jax.sharding import Mesh, PartitionSpec
        from jax.experimental.shard_map import shard_map
        from concourse import bass2jax, mybir as mb

        bass2jax.install_neuronx_cc_hook()
        self.jax = jax
        self.nc = nc
        partition_name = (
            nc.partition_id_tensor.name if nc.partition_id_tensor else None
        )
        in_names, out_names, out_avals = [], [], []
        for alloc in nc.m.functions[0].allocations:
            if not isinstance(alloc, mb.MemoryLocationSet):
                continue
            name = alloc.memorylocations[0].name
            if alloc.kind == "ExternalInput":
                if name != partition_name:
                    in_names.append(name)
            elif alloc.kind == "ExternalOutput":
                out_names.append(name)
                out_avals.append(
                    jax.core.ShapedArray(
                        tuple(alloc.tensor_shape), mb.dt.np(alloc.dtype)
                    )
                )
        self.n_params = len(in_names)
        self.out_names = out_names
        self.out_avals = out_avals
        self.zero_outs = [
            np.zeros((N_CORES * a.shape[0], *a.shape[1:]), a.dtype)
            for a in out_avals
        ]
        all_in_names = list(in_names) + list(out_names)
        if partition_name is not None:
            all_in_names.append(partition_name)
        self.in_names = in_names

        def _body(*args):
            operands = list(args)
            if partition_name is not None:
                operands.append(bass2jax.partition_id_tensor())
            outs = bass2jax._bass_exec_p.bind(
                *operands,
                out_avals=tuple(out_avals),
                in_names=tuple(all_in_names),
                out_names=tuple(out_names),
                lowering_input_output_aliases=(),
                sim_require_finite=True,
                sim_require_nnan=True,
                nc=nc,
            )
            return tuple(outs)

        devices = jax.devices()[:N_CORES]
        mesh = Mesh(np.asarray(devices), ("core",))
        n_all = self.n_params + len(out_names)
        self.fn = jax.jit(
            shard_map(
                _body,
                mesh=mesh,
                in_specs=(PartitionSpec("core"),) * n_all,
                out_specs=(PartitionSpec("core"),) * len(out_names),
                check_rep=False,
            ),
            keep_unused=True,
        )

    def stage(self, in_maps):
        """Concatenate per-core inputs along axis 0 (host-side)."""
        concat = [
            np.concatenate([np.asarray(m[n]) for m in in_maps], axis=0)
            for n in self.in_names
        ]
        return concat + self.zero_outs

    def run_staged(self, staged):
        return self.fn(*staged)

    def __call__(self, in_maps):
        out_arrs = self.fn(*self.stage(in_maps))
        return [
            {
                n: np.asarray(out_arrs[i]).reshape(
                    N_CORES, *self.out_avals[i].shape
                )[c]
                for i, n in enumerate(self.out_names)
            }
            for c in range(N_CORES)
        ]


def _get_runner(reps: int = 1):
    key = f"runner{reps}"
    if key not in _CACHE:
        _CACHE[key] = _Runner(_get_program(reps))
    return _CACHE[key]


def _to_chunked_fp8(arr_t: np.ndarray) -> np.ndarray:
    """[HID, cols] -> fp8 [128, KTI*cols] with din chunk-major free dim."""
    cols = arr_t.shape[1]
    return np.ascontiguousarray(
        arr_t.reshape(KTI, 128, cols).transpose(1, 0, 2).reshape(128, KTI * cols)
    ).astype(F8_NP)


def _w_to_tiled_fp8(w_t: np.ndarray) -> np.ndarray:
    """[din, dout] -> fp8 [128, DT*KTI*128], dout-tile-major: block t holds
    the [128, KTI, 128] din-chunked weights for dout tile t."""
    return np.ascontiguousarray(
        w_t.reshape(KTI, 128, DT, 128)
        .transpose(1, 2, 0, 3)
        .reshape(128, DT * KTI * 128)
    ).astype(F8_NP)


def make_in_maps(attention_mask, query, key, value, Wq, bq, Wk, bk):
    """Host-side shard + layout prep. Returns per-core input dicts."""
    attention_mask = np.asarray(attention_mask, dtype=np.float32)
    query = np.asarray(query, dtype=np.float32)
    key = np.asarray(key, dtype=np.float32)
    value = np.asarray(value, dtype=np.float32)
    Wq = np.asarray(Wq, dtype=np.float32)
    bq = np.asarray(bq, dtype=np.float32)
    Wk = np.asarray(Wk, dtype=np.float32)
    bk = np.asarray(bk, dtype=np.float32)

    scale = 1.0 / np.sqrt(np.float32(HD))
    wq8 = _w_to_tiled_fp8(np.ascontiguousarray(Wq.T) * W_SCALE)  # [din, dout]
    wk8 = _w_to_tiled_fp8(np.ascontiguousarray(Wk.T) * W_SCALE)
    # interleave per dout tile: [wq_t | wk_t]
    w8 = np.concatenate(
        [
            np.concatenate(
                [wq8[:, t * KTI * 128 : (t + 1) * KTI * 128],
                 wk8[:, t * KTI * 128 : (t + 1) * KTI * 128]],
                axis=1,
            )
            for t in range(DT)
        ],
        axis=1,
    )
    bq_t = np.ascontiguousarray((bq * scale).reshape(DT, 128).T).astype(np.float32)
    bk_t = np.ascontiguousarray(bk.reshape(DT, 128).T).astype(np.float32)
    consts = np.concatenate(
        [bq_t, bk_t, np.eye(128, dtype=np.float32)], axis=1
    ).astype(np.float32)

    in_maps = []
    for core in range(N_CORES):
        b, qh = divmod(core, 2)
        q0 = qh * SQ
        q8 = _to_chunked_fp8(np.ascontiguousarray(query[b, q0 : q0 + SQ, :].T))
        k8 = _to_chunked_fp8(np.ascontiguousarray(key[b].T))
        w = np.exp(attention_mask[b, 0, 0, :]).astype(np.float32) / np.float32(NH)
        vw = (value[b] * w[:, None]).astype(np.float32)  # [S, HID]
        v_in = np.ascontiguousarray(
            vw.reshape(KTI, 128, HID).transpose(1, 0, 2).reshape(128, KTI * HID)
        ).astype(BF16_NP)
        in_maps.append(
            {
                "q8_in": q8,
                "k8_in": k8,
                "w8_in": w8,
                "v_in": v_in,
                "const_in": consts,
            }
        )
    return in_maps


def gather_output(results):
    out = np.empty((B, S, HID), dtype=np.float32)
    for core in range(N_CORES):
        b, qh = divmod(core, 2)
        q0 = qh * SQ
        out[b, q0 : q0 + SQ, :] = results[core]["o_out"]
    return out


def kernel(attention_mask, query, key, value, Wq, bq, Wk, bk):
    runner = _get_runner()
    in_maps = make_in_maps(attention_mask, query, key, value, Wq, bq, Wk, bk)
    return gather_output(runner(in_maps))


if __name__ == "__main__":
    rng = np.random.default_rng(0)
    ins = {
        "attention_mask": np.zeros((B, 1, 1, S), np.float32),
        "query": rng.standard_normal((B, S, HID)).astype(np.float32),
        "key": rng.standard_normal((B, S, HID)).astype(np.float32),
        "value": rng.standard_normal((B, S, HID)).astype(np.float32),
        "Wq": (rng.standard_normal((HID, HID)) * 0.02).astype(np.float32),
        "bq": np.zeros(HID, np.float32),
        "Wk": (rng.standard_normal((HID, HID)) * 0.02).astype(np.float32),
        "bk": np.zeros(HID, np.float32),
    }
    out = kernel(**ins)
    print("kernel output:", out.shape, out.dtype)


# revision 35
# speedup vs baseline: 5.3380x; 5.3380x over previous
"""AttentionTeacher Trainium2 kernel (fp8 DoubleRow projections +
engine-rebalanced softmax pipeline).

Math (reference):
    q = query @ Wq.T + bq;  k = key @ Wk.T + bk          [B,S,HID]
    per head h (HD=64): scores_h = q_h k_h^T / 8 + mask  [B,NH,S,S]
    probs_h = softmax(scores_h)
    out = (sum_h probs_h) @ V / NH                       [B,S,HID]

Sharding: 8 cores, SPMD, no collectives. Core i handles batch b=i//2 and
query rows [512*(i%2), 512*(i%2+1)). The K-projection is duplicated
across the pair of cores sharing a batch (cheap in fp8: ~3.4us of PE).

The critical engine is ACT: 64 exp([128,1024]) ops with accum_out row
sums are ~78us and exp runs nowhere else, so ACT does *only* exp (plus
evacuations for t=0 during the DMA ramp and the final qblock's tail
copies, both in ACT idle windows). Everything else is balanced around
that wall (TimelineSim busy, per core):
  PE   ~60us: fp8e4 DoubleRow projections (2 k-planes/instr, 0.5
       cyc/col), bf16 scores (K=64 via tile_position row pairs), fp32
       transposes of P, bf16 P^T @ V.
  DVE  ~77us: all PSUM evacuations (tensor_scalar with scale+bias
       folded), even-head chain STTs, odd-head prescales, reciprocals,
       qb<3 copies, the qb3 tail merge.
  Pool ~62us: odd-head chain adds (plain TensorTensor: GPSIMD can read
       neither PSUM nor run TensorScalarPtr on real HW), qb<3 merges.

fp8 notes: W is staged *32 (avoids e4m3 denormals at sigma=0.02); the
1/32 (and the 1/8 score scale on the q side) is folded into the
evacuation. Scores/PV stay bf16: fp8 P would sit in the denormal range
and the extra score error would eat the 2e-2 budget (measured rel err
6.4e-3 vs 2.5e-3 all-bf16).

Schedule notes (emission order ~ per-engine execution order):
  - One serial HBM stream in the model: DMAs are criticality-ordered
    (biases, tile-0 W, kin, qin, W tiles 1-7, ident, V) and sized so the
    ~650ns/DMA fixed cost stays amortized; K-projection is emitted
    before Q so PE follows the DMA arrival order.
  - Chains lag exps by one head pair (per-pair reciprocals); the final
    pair gets per-head reciprocals so chain 14 overlaps exp 15, and
    qb3's Pool sub-chain ends at h13 so only h15's STT and the halved
    DVE merge separate the last exp from the transposes.
  - Each phase prefetches the next qblock's first head pair ahead of the
    end-of-phase pace work; qb0/qb1/qb2 outputs are stage-split
    (transposes / PV half / PV half) between head pairs, with qb2's last
    PV half after the final exp to keep PE p-state warm into qb3's PV.
  - pp/ov live in separate single-buffer PSUM pools so the scheduler
    doesn't serialize qb2's ready PV behind qb3's transposes.
"""

import numpy as np
import ml_dtypes

import concourse.bass as bass
import concourse.tile as tile
from concourse import bacc, mybir
from concourse.bass_utils import run_bass_kernel_spmd

N_CORES = 8
B, S, HID, NH, HD = 4, 1024, 1024, 16, 64
SQ = S // 2          # query rows per core
QB = SQ // 128       # query blocks per core
DT = HID // 128      # dout tiles (2 heads each)
KTI = HID // 128     # contraction (din) tiles
CD = mybir.dt.bfloat16
F8 = mybir.dt.float8e4
F32 = mybir.dt.float32
BF16_NP = ml_dtypes.bfloat16
F8_NP = mybir.dt.np(F8)

W_SCALE = 32.0       # host-side W upscale (fp8 denormal avoidance)
Q_EVAC = 1.0 / (W_SCALE * 8.0)   # un-scale + 1/sqrt(HD)
K_EVAC = 1.0 / W_SCALE

_ts = bass.ts
_mult = mybir.AluOpType.mult
_add = mybir.AluOpType.add
_EXP = mybir.ActivationFunctionType.Exp
_IDENT = mybir.ActivationFunctionType.Identity
_DR = mybir.MatmulPerfMode.DoubleRow

_CACHE: dict = {}

# Real-HW engine limits (BIR verifier): Pool/GPSIMD cannot read PSUM and
# cannot run TensorScalarPtr. So: PSUM evacuations live on DVE (plus the
# first tiles on ACT, which is otherwise idle during the DMA ramp), and
# Pool chain heads are fed by a DVE prescale (E *= 1/Z in place) followed
# by a plain Pool TensorTensor add.
_DVE_HEADS = frozenset(range(0, NH, 2))  # one Pool head per pair
_ACT_EVAC_T = frozenset({0})


def _build_program(reps: int = 1):
    nc = bacc.Bacc(
        "TRN2", target_bir_lowering=False, debug=False, num_devices=N_CORES
    )
    d_q8 = nc.dram_tensor("q8_in", [128, KTI * SQ], F8, kind="ExternalInput")
    d_k8 = nc.dram_tensor("k8_in", [128, KTI * S], F8, kind="ExternalInput")
    # W interleaved per dout tile: block t = [wq_t | wk_t], each [128, KTI*128]
    d_w8 = nc.dram_tensor(
        "w8_in", [128, 2 * KTI * HID], F8, kind="ExternalInput"
    )
    d_v = nc.dram_tensor("v_in", [128, KTI * HID], CD, kind="ExternalInput")
    # consts merged: bq [0:DT], bk [DT:2DT], ident [2DT:2DT+128]
    d_c = nc.dram_tensor("const_in", [128, 2 * DT + 128], F32, kind="ExternalInput")
    d_o = nc.dram_tensor("o_out", [SQ, HID], F32, kind="ExternalOutput")

    with tile.TileContext(nc) as tc:
        with (
            tc.tile_pool(name="const", bufs=1) as const_pool,
            tc.tile_pool(name="xin", bufs=1) as xin_pool,
            tc.tile_pool(name="proj", bufs=1) as proj_pool,
            tc.tile_pool(name="e", bufs=20) as e_pool,
            tc.tile_pool(name="ch", bufs=8) as ch_pool,
            tc.tile_pool(name="z", bufs=20) as z_pool,
            tc.tile_pool(name="pt", bufs=2) as pt_pool,
            tc.tile_pool(name="osb", bufs=2) as o_pool,
            tc.tile_pool(name="proj_ps", bufs=2, space="PSUM") as proj_ps,
            tc.tile_pool(name="sc_ps", bufs=2, space="PSUM") as sc_ps,
            tc.tile_pool(name="pp_ps", bufs=1, space="PSUM") as pp_ps,
            tc.tile_pool(name="ov_ps", bufs=1, space="PSUM") as ov_ps,
        ):
          for _rep in range(reps):
            # ---- input DMAs (criticality-ordered; big transfers so the
            # per-DMA HWDGE fixed cost (~650ns) doesn't dominate) ----
            w_sb = xin_pool.tile([128, 2 * KTI * HID], F8, tag="w8", name="w8")
            qin_sb = xin_pool.tile([128, KTI * SQ], F8, tag="q8", name="q8")
            kin_sb = xin_pool.tile([128, KTI * S], F8, tag="k8", name="k8")
            c_sb = const_pool.tile([128, 2 * DT + 128], F32, tag="c", name="c_sb")
            nc.sync.dma_start(c_sb[:], d_c.ap()[:])
            nc.sync.dma_start(
                w_sb[:, _ts(0, 2 * HID)], d_w8.ap()[:, _ts(0, 2 * HID)]
            )
            nc.sync.dma_start(kin_sb[:], d_k8.ap()[:])
            nc.sync.dma_start(qin_sb[:], d_q8.ap()[:])
            for t in range(1, DT):
                nc.sync.dma_start(
                    w_sb[:, _ts(t, 2 * HID)], d_w8.ap()[:, _ts(t, 2 * HID)]
                )
            v_sb = xin_pool.tile([128, KTI * HID], CD, tag="v", name="v_sb")
            nc.sync.dma_start(v_sb[:], d_v.ap()[:])

            bq_sb = c_sb[:, 0:DT]
            bk_sb = c_sb[:, DT : 2 * DT]
            ident = c_sb[:, 2 * DT : 2 * DT + 128]
            # per-tile [128, KTI, 128] views of W; [128, KTI, cols] of q/k/v
            wq3 = [
                w_sb[:, _ts(2 * t, HID)].rearrange("p (c f) -> p c f", c=KTI)
                for t in range(DT)
            ]
            wk3 = [
                w_sb[:, _ts(2 * t + 1, HID)].rearrange("p (c f) -> p c f", c=KTI)
                for t in range(DT)
            ]
            qin3 = qin_sb[:].rearrange("p (c f) -> p c f", c=KTI)
            kin3 = kin_sb[:].rearrange("p (c f) -> p c f", c=KTI)
            v3 = v_sb[:].rearrange("p (c f) -> p c f", c=KTI)

            # preload the ACT exp table while DMAs run
            warm = const_pool.tile([128, 1], F32, tag="warm", name="warm")
            nc.gpsimd.memset(warm[:], 0.0)
            warm2 = const_pool.tile([128, 1], F32, tag="warm2", name="warm2")
            nc.scalar.activation(warm2[:], warm[:], _EXP)

            qt = [
                proj_pool.tile([128, SQ], CD, tag=f"qt{t}", name=f"qt{t}")
                for t in range(DT)
            ]
            ktp = [
                proj_pool.tile([128, S], CD, tag=f"kt{t}", name=f"ktp{t}")
                for t in range(DT)
            ]

            # ---- per-qblock attention state ----
            zts = {}     # qb -> [128, NH] f32 row sums
            es = {}      # (qb, h) -> E tile (f32)
            invz = {}    # (qb, h) -> [128, 1] f32 reciprocal column
            chain_a = {}  # qb -> DVE sub-chain tile
            chain_b = {}  # qb -> Pool sub-chain tile
            merged = {}  # qb -> P tile

            def emit_proj(t):
                # K first: kin is DMA'd before qin, and PE runs in FIFO order
                for nh in range(2):
                    ps2 = proj_ps.tile([128, 512], F32, tag="proj", name="proj_k_ps")
                    for j in range(KTI // 2):
                        nc.tensor.matmul(
                            ps2[:],
                            wk3[t][:, 2 * j : 2 * j + 2, :],
                            kin3[:, 2 * j : 2 * j + 2, _ts(nh, 512)],
                            start=(j == 0), stop=(j == KTI // 2 - 1),
                            perf_mode=_DR,
                        )
                    if t in _ACT_EVAC_T:
                        nc.scalar.activation(
                            ktp[t][:, _ts(nh, 512)], ps2[:], _IDENT,
                            bias=bk_sb[:, t : t + 1], scale=K_EVAC,
                        )
                    else:
                        nc.vector.tensor_scalar(
                            out=ktp[t][:, _ts(nh, 512)], in0=ps2[:], scalar1=K_EVAC,
                            scalar2=bk_sb[:, t : t + 1], op0=_mult, op1=_add,
                        )
                ps = proj_ps.tile([128, SQ], F32, tag="proj", name="proj_q_ps")
                for j in range(KTI // 2):
                    nc.tensor.matmul(
                        ps[:],
                        wq3[t][:, 2 * j : 2 * j + 2, :],
                        qin3[:, 2 * j : 2 * j + 2, :],
                        start=(j == 0), stop=(j == KTI // 2 - 1),
                        perf_mode=_DR,
                    )
                if t in _ACT_EVAC_T:
                    nc.scalar.activation(
                        qt[t][:], ps[:], _IDENT,
                        bias=bq_sb[:, t : t + 1], scale=Q_EVAC,
                    )
                else:
                    nc.vector.tensor_scalar(
                        out=qt[t][:], in0=ps[:], scalar1=Q_EVAC,
                        scalar2=bq_sb[:, t : t + 1], op0=_mult, op1=_add,
                    )

            def emit_head(qb, h):
                if qb not in zts:
                    zts[qb] = z_pool.tile([128, NH], F32, tag="z", name="zt")
                t, half = h // 2, h % 2
                d0 = 64 * half
                sc = sc_ps.tile([128, S], F32, tag="sc", name="sc")
                for n2 in range(2):
                    nc.tensor.matmul(
                        sc[:, _ts(n2, 512)],
                        qt[t][d0 : d0 + 64, _ts(qb, 128)],
                        ktp[t][d0 : d0 + 64, _ts(n2, 512)],
                        start=True, stop=True, tile_position=(d0, 0),
                    )
                e = e_pool.tile([128, S], F32, tag="e", name="e")
                nc.scalar.activation(e[:], sc[:], _EXP, accum_out=zts[qb][:, h : h + 1])
                es[(qb, h)] = e

            def emit_recip(qb, h0, nh):
                # inverse of Z for heads [h0, h0+nh) right after their exps
                inv = z_pool.tile([128, nh], F32, tag="z", name="inv_z")
                nc.vector.reciprocal(inv[:], zts[qb][:, h0 : h0 + nh])
                for k in range(nh):
                    invz[(qb, h0 + k)] = inv[:, k : k + 1]

            def emit_chain(qb, h):
                # DVE-native heads: one STT. Pool heads: DVE prescale of E
                # in place, then a plain TT add on Pool.
                sv = invz[(qb, h)]
                e = es[(qb, h)]
                on_dve = h in _DVE_HEADS or (qb == 3 and h == 15)
                book = chain_a if on_dve else chain_b
                prev = book.get(qb)
                if on_dve:
                    nxt = ch_pool.tile([128, S], F32, tag="ch", name="chain")
                    if prev is None:
                        nc.vector.tensor_scalar(
                            out=nxt[:], in0=e[:], scalar1=sv,
                            scalar2=None, op0=_mult,
                        )
                    else:
                        nc.vector.scalar_tensor_tensor(
                            out=nxt[:], in0=e[:], scalar=sv,
                            in1=prev[:], op0=_mult, op1=_add,
                        )
                    book[qb] = nxt
                else:
                    nc.vector.tensor_scalar(
                        out=e[:], in0=e[:], scalar1=sv, scalar2=None, op0=_mult
                    )
                    if prev is None:
                        book[qb] = e  # scaled E doubles as the B seed
                    else:
                        nxt = ch_pool.tile([128, S], F32, tag="ch", name="chain")
                        nc.gpsimd.tensor_tensor(
                            out=nxt[:], in0=e[:], in1=prev[:], op=_add
                        )
                        book[qb] = nxt

            def emit_merge(qb):
                out = ch_pool.tile([128, S], F32, tag="ch", name="pmerged")
                if qb < 3:
                    nc.gpsimd.tensor_tensor(
                        out=out[:], in0=chain_a[qb][:], in1=chain_b[qb][:], op=_add
                    )
                else:
                    # tail merge on DVE in halves (qb3's Pool B-chain ends
                    # at h13, so only h15's STT separates exp15 from this)
                    for c in range(2):
                        nc.vector.tensor_tensor(
                            out=out[:, _ts(c, 512)],
                            in0=chain_a[3][:, _ts(c, 512)],
                            in1=chain_b[3][:, _ts(c, 512)], op=_add,
                        )
                merged[qb] = out

            def emit_pace(qb, i):
                # chains lag exps by one pair; final pair gets per-head
                # recips so chain 14 overlaps exp 15
                if i == 7:
                    emit_recip(qb, 14, 1)
                    emit_chain(qb, 12)
                    emit_chain(qb, 13)
                    emit_chain(qb, 14)
                    emit_recip(qb, 15, 1)
                    emit_chain(qb, 15)
                    emit_merge(qb)
                    return
                emit_recip(qb, 2 * i, 2)
                if i >= 1:
                    emit_chain(qb, 2 * i - 2)
                    emit_chain(qb, 2 * i - 1)

            out_state = {}

            def emit_out_stage(qb, stage):
                # staged so each piece of PE work interleaves between head
                # pairs instead of starving the exp stream
                if stage == 0:
                    # PE transposes of P (f32) -> bf16 P^T via DVE/Pool
                    p = merged[qb]
                    ptsb = pt_pool.tile([128, S], CD, tag="pt", name="ptsb")
                    for c in range(2):
                        pp = pp_ps.tile([128, 512], F32, tag="pp", name="pp")
                        for j in range(4):
                            nc.tensor.transpose(
                                pp[:, _ts(j, 128)], p[:, _ts(4 * c + j, 128)],
                                ident,
                            )
                        if qb == 3:
                            nc.scalar.copy(ptsb[:, _ts(c, 512)], pp[:])
                        else:
                            nc.vector.tensor_copy(ptsb[:, _ts(c, 512)], pp[:])
                    osb = o_pool.tile([128, HID], F32, tag="osb", name="osb")
                    out_state[qb] = (ptsb, osb)
                else:
                    n2 = stage - 1
                    ptsb, osb = out_state[qb]
                    ov = ov_ps.tile([128, 512], F32, tag="ov", name="ov")
                    for kt_i in range(KTI):
                        nc.tensor.matmul(
                            ov[:], ptsb[:, _ts(kt_i, 128)],
                            v3[:, kt_i, _ts(n2, 512)],
                            start=(kt_i == 0), stop=(kt_i == KTI - 1),
                        )
                    if qb == 3:
                        nc.scalar.copy(osb[:, _ts(n2, 512)], ov[:])
                    else:
                        nc.vector.tensor_copy(osb[:, _ts(n2, 512)], ov[:])
                    nc.sync.dma_start(
                        d_o.ap()[_ts(qb, 128), _ts(n2, 512)], osb[:, _ts(n2, 512)]
                    )

            def emit_out(qb):
                for stage in range(3):
                    emit_out_stage(qb, stage)

            # ---- emission schedule ----
            # t-loop: heads for tile t, next tile's projections (so their
            # evacuations precede chain ops in the DVE/Pool queues), pacing.
            # Each phase prefetches the next qblock's first head pair ahead
            # of the heavy end-of-phase pace (chains+merge) so the exp
            # stream doesn't stall at the transition.
            emit_proj(0)
            for t in range(DT):
                for qb in (0, 1):
                    emit_head(qb, 2 * t)
                    emit_head(qb, 2 * t + 1)
                if t + 1 < DT:
                    emit_proj(t + 1)
                if t == DT - 1:
                    emit_head(2, 0)
                    emit_head(2, 1)
                emit_pace(0, t)
                emit_pace(1, t)
            # qb2 phase (pair i emitted during pace i-1 slot)
            out_sched_2 = {2: (0, 0), 3: (0, 1), 4: (0, 2), 5: (1, 0), 6: (1, 1)}
            for i in range(1, 8):
                emit_head(2, 2 * i)
                emit_head(2, 2 * i + 1)
                emit_pace(2, i - 1)
                if i - 1 in out_sched_2:
                    emit_out_stage(*out_sched_2[i - 1])
            emit_head(3, 0)
            emit_head(3, 1)
            emit_head(3, 2)
            emit_head(3, 3)
            emit_pace(2, 7)
            emit_pace(3, 0)
            emit_out_stage(1, 2)
            # qb3 phase: qb2's PV held late so PE stays warm into the tail
            out_sched_3 = {5: (2, 0), 6: (2, 1)}
            for i in range(2, 8):
                emit_head(3, 2 * i)
                emit_head(3, 2 * i + 1)
                emit_pace(3, i - 1)
                if i - 1 in out_sched_3:
                    emit_out_stage(*out_sched_3[i - 1])
            emit_pace(3, 7)
            emit_out_stage(2, 2)
            emit_out(3)

    nc.compile()
    return nc


def _get_program(reps: int = 1):
    key = f"nc{reps}"
    if key not in _CACHE:
        _CACHE[key] = _build_program(reps)
    return _CACHE[key]


class _Runner:
    """Compile-once SPMD executor (mirrors run_bass_via_pjrt's multi-core
    path, but keeps the jitted function so repeat calls skip re-compile)."""

    def __init__(self, nc):
        import jax
        from jax.sharding import Mesh, PartitionSpec
        from jax.experimental.shard_map import shard_map
        from concourse import bass2jax, mybir as mb

        bass2jax.install_neuronx_cc_hook()
        self.jax = jax
        self.nc = nc
        partition_name = (
            nc.partition_id_tensor.name if nc.partition_id_tensor else None
        )
        in_names, out_names, out_avals = [], [], []
        for alloc in nc.m.functions[0].allocations:
            if not isinstance(alloc, mb.MemoryLocationSet):
                continue
            name = alloc.memorylocations[0].name
            if alloc.kind == "ExternalInput":
                if name != partition_name:
                    in_names.append(name)
            elif alloc.kind == "ExternalOutput":
                out_names.append(name)
                out_avals.append(
                    jax.core.ShapedArray(
                        tuple(alloc.tensor_shape), mb.dt.np(alloc.dtype)
                    )
                )
        self.n_params = len(in_names)
        self.out_names = out_names
        self.out_avals = out_avals
        self.zero_outs = [
            np.zeros((N_CORES * a.shape[0], *a.shape[1:]), a.dtype)
            for a in out_avals
        ]
        all_in_names = list(in_names) + list(out_names)
        if partition_name is not None:
            all_in_names.append(partition_name)
        self.in_names = in_names

        def _body(*args):
            operands = list(args)
            if partition_name is not None:
                operands.append(bass2jax.partition_id_tensor())
            outs = bass2jax._bass_exec_p.bind(
                *operands,
                out_avals=tuple(out_avals),
                in_names=tuple(all_in_names),
                out_names=tuple(out_names),
                lowering_input_output_aliases=(),
                sim_require_finite=True,
                sim_require_nnan=True,
                nc=nc,
            )
            return tuple(outs)

        devices = jax.devices()[:N_CORES]
        mesh = Mesh(np.asarray(devices), ("core",))
        n_all = self.n_params + len(out_names)
        self.fn = jax.jit(
            shard_map(
                _body,
                mesh=mesh,
                in_specs=(PartitionSpec("core"),) * n_all,
                out_specs=(PartitionSpec("core"),) * len(out_names),
                check_rep=False,
            ),
            keep_unused=True,
        )

    def stage(self, in_maps):
        """Concatenate per-core inputs along axis 0 (host-side)."""
        concat = [
            np.concatenate([np.asarray(m[n]) for m in in_maps], axis=0)
            for n in self.in_names
        ]
        return concat + self.zero_outs

    def run_staged(self, staged):
        return self.fn(*staged)

    def __call__(self, in_maps):
        out_arrs = self.fn(*self.stage(in_maps))
        return [
            {
                n: np.asarray(out_arrs[i]).reshape(
                    N_CORES, *self.out_avals[i].shape
                )[c]
                for i, n in enumerate(self.out_names)
            }
            for c in range(N_CORES)
        ]


def _get_runner(reps: int = 1):
    key = f"runner{reps}"
    if key not in _CACHE:
        _CACHE[key] = _Runner(_get_program(reps))
    return _CACHE[key]


def _to_chunked_fp8(arr_t: np.ndarray) -> np.ndarray:
    """[HID, cols] -> fp8 [128, KTI*cols] with din chunk-major free dim."""
    cols = arr_t.shape[1]
    return np.ascontiguousarray(
        arr_t.reshape(KTI, 128, cols).transpose(1, 0, 2).reshape(128, KTI * cols)
    ).astype(F8_NP)


def _w_to_tiled_fp8(w_t: np.ndarray) -> np.ndarray:
    """[din, dout] -> fp8 [128, DT*KTI*128], dout-tile-major: block t holds
    the [128, KTI, 128] din-chunked weights for dout tile t."""
    return np.ascontiguousarray(
        w_t.reshape(KTI, 128, DT, 128)
        .transpose(1, 2, 0, 3)
        .reshape(128, DT * KTI * 128)
    ).astype(F8_NP)


def make_in_maps(attention_mask, query, key, value, Wq, bq, Wk, bk):
    """Host-side shard + layout prep. Returns per-core input dicts."""
    attention_mask = np.asarray(attention_mask, dtype=np.float32)
    query = np.asarray(query, dtype=np.float32)
    key = np.asarray(key, dtype=np.float32)
    value = np.asarray(value, dtype=np.float32)
    Wq = np.asarray(Wq, dtype=np.float32)
    bq = np.asarray(bq, dtype=np.float32)
    Wk = np.asarray(Wk, dtype=np.float32)
    bk = np.asarray(bk, dtype=np.float32)

    scale = 1.0 / np.sqrt(np.float32(HD))
    wq8 = _w_to_tiled_fp8(np.ascontiguousarray(Wq.T) * W_SCALE)  # [din, dout]
    wk8 = _w_to_tiled_fp8(np.ascontiguousarray(Wk.T) * W_SCALE)
    # interleave per dout tile: [wq_t | wk_t]
    w8 = np.concatenate(
        [
            np.concatenate(
                [wq8[:, t * KTI * 128 : (t + 1) * KTI * 128],
                 wk8[:, t * KTI * 128 : (t + 1) * KTI * 128]],
                axis=1,
            )
            for t in range(DT)
        ],
        axis=1,
    )
    bq_t = np.ascontiguousarray((bq * scale).reshape(DT, 128).T).astype(np.float32)
    bk_t = np.ascontiguousarray(bk.reshape(DT, 128).T).astype(np.float32)
    consts = np.concatenate(
        [bq_t, bk_t, np.eye(128, dtype=np.float32)], axis=1
    ).astype(np.float32)

    in_maps = []
    for core in range(N_CORES):
        b, qh = divmod(core, 2)
        q0 = qh * SQ
        q8 = _to_chunked_fp8(np.ascontiguousarray(query[b, q0 : q0 + SQ, :].T))
        k8 = _to_chunked_fp8(np.ascontiguousarray(key[b].T))
        w = np.exp(attention_mask[b, 0, 0, :]).astype(np.float32) / np.float32(NH)
        vw = (value[b] * w[:, None]).astype(np.float32)  # [S, HID]
        v_in = np.ascontiguousarray(
            vw.reshape(KTI, 128, HID).transpose(1, 0, 2).reshape(128, KTI * HID)
        ).astype(BF16_NP)
        in_maps.append(
            {
                "q8_in": q8,
                "k8_in": k8,
                "w8_in": w8,
                "v_in": v_in,
                "const_in": consts,
            }
        )
    return in_maps


def gather_output(results):
    out = np.empty((B, S, HID), dtype=np.float32)
    for core in range(N_CORES):
        b, qh = divmod(core, 2)
        q0 = qh * SQ
        out[b, q0 : q0 + SQ, :] = results[core]["o_out"]
    return out


def kernel(attention_mask, query, key, value, Wq, bq, Wk, bk):
    runner = _get_runner()
    in_maps = make_in_maps(attention_mask, query, key, value, Wq, bq, Wk, bk)
    return gather_output(runner(in_maps))


if __name__ == "__main__":
    rng = np.random.default_rng(0)
    ins = {
        "attention_mask": np.zeros((B, 1, 1, S), np.float32),
        "query": rng.standard_normal((B, S, HID)).astype(np.float32),
        "key": rng.standard_normal((B, S, HID)).astype(np.float32),
        "value": rng.standard_normal((B, S, HID)).astype(np.float32),
        "Wq": (rng.standard_normal((HID, HID)) * 0.02).astype(np.float32),
        "bq": np.zeros(HID, np.float32),
        "Wk": (rng.standard_normal((HID, HID)) * 0.02).astype(np.float32),
        "bk": np.zeros(HID, np.float32),
    }
    out = kernel(**ins)
    print("kernel output:", out.shape, out.dtype)
